# revision 34
# baseline (speedup 1.0000x reference)
"""DecoderAttentionSingle Trainium2 Bass kernel (v3).

8 NeuronCores, pure batch-parallel: one [C,H,W] image per core.

Per-core dataflow (bf16 data, fp32 PSUM):
  k2 = [W_enc|W_enc]^T enc (PE, output duplicated on both partition halves)
       -> kpad2b [128, 130, 130]: half1 k at col offset 1, half2 at col
       offset 0 (pre-shifted so one 128-partition add evaluates the
       neighbor PAIR (d,-1),(d,0) at once).
  q2 = [W_dec|W_dec]^T dec + bsum (PE, duplicated) -> q2c.
  scores: pair adds on GpSimd (32-row granularity, 3 ops), single adds on
      DVE; tanh on ACT; PE pair-dot matmuls -> dps [10, 4*W] PSUM; DVE
      drain -> sc_sb.
  XBAR dma transpose sc_sb -> e_pm [128(w), 16, 16(n)] pixel-major.
  softmax: exp (ACT), mask/reduce/recip (DVE), normalize into x2-duplicated
      e_pm2 [128, 16, 16, 2] (innermost real pair keeps DVE at 2 elem/cyc).
  MAC: 9 DVE mults (prod = e*ent); 8 accumulations via SWDGE DMA with
      accum_op=add into accpm (CCE adder in the DMA datapath); encoder
      neighborhoods via XBAR transposes from DRAM (w-shift = +-1 element
      offset; wrap garbage masked).
  conv3x3 (PE, 9 matmuls x 2-window PSUM chains) -> vals_pc parity-packed.
  attn XBAR -> attn_pc [(h%2,c), h2, w]; out = W2v^T vals + W2a^T attn
      (block-diag weights); ACT Lrelu drain (bias+leaky fused); DMA store.
"""

import sys

sys.path.insert(0, "/opt/trn_rl_repo")

from contextlib import ExitStack

import ml_dtypes
import numpy as np

import concourse.bass as bass
import concourse.mybir as mybir
import concourse.tile as tile
from concourse import bacc
from concourse.bass_utils import run_bass_kernel_spmd

BF16 = mybir.dt.bfloat16
FP32 = mybir.dt.float32
AF = mybir.ActivationFunctionType
ALU = mybir.AluOpType

B, ENC, DEC, H, W = 8, 64, 128, 128, 128
N_CORES = 8
HP, WP = H + 2, W + 2

OFFS = [(dr, dc) for dr in (-1, 0, 1) for dc in (-1, 0, 1)]
# dps/sc_sb row j holds neighbor NMAP[j]
NMAP = [0, 1, 3, 4, 6, 7, 2, 5, 8]

RC = 16            # rows per softmax/MAC chunk
RC2 = 32           # rows per pair-add chunk (2 chunks)
NCH = H // RC      # 8 chunks
KG = 4             # rows per k/q PSUM group
ENCF_PAD = 256     # front/back zero pad (elements) of flat enc DRAM image
ENCF_N = 2 * ENCF_PAD + H * W

LRELU_ACT = [False]    # HW Lrelu ignores alpha (plain relu); keep DVE fallback
MAC_DMA_ACCUM = [False]

# constb packed offsets (bf16 [128, CONSTB_N])
OFF_WENC2 = 0                   # [64(rows used), 128] W_enc duplicated cols
OFF_WDEC2 = 128                 # [128, 128] W_dec duplicated cols
OFF_WAGG5 = 256                 # [128, 5*10]
OFF_CONVW = 306                 # [128, 9*64]
OFF_W2V = 882                   # [128, 128] block-diag vals half of W_attn
OFF_W2A = 1010                  # [128, 128] block-diag attn half of W_attn
OFF_MASK = 1138                 # [128, 128*16] pixel-major mask
CONSTB_N = OFF_MASK + H * 16


def build_program():
    nc = bacc.Bacc(None, target_bir_lowering=False, debug=False)

    encf_d = nc.dram_tensor("encf", [ENC, ENCF_N], BF16, kind="ExternalInput").ap()
    decp_d = nc.dram_tensor("decp", [DEC, HP, WP], BF16, kind="ExternalInput").ap()
    cb_d = nc.dram_tensor("constb", [128, CONSTB_N], BF16, kind="ExternalInput").ap()
    cf_d = nc.dram_tensor("constf", [128, 4], FP32, kind="ExternalInput").ap()
    out_d = nc.dram_tensor("out", [ENC, H, W], BF16, kind="ExternalOutput").ap()

    with tile.TileContext(nc) as tc, ExitStack() as ctx:
        const = ctx.enter_context(tc.tile_pool(name="const", bufs=1))
        big = ctx.enter_context(tc.tile_pool(name="big", bufs=1))
        encp = ctx.enter_context(tc.tile_pool(name="encp", bufs=3))
        q2p = ctx.enter_context(tc.tile_pool(name="q2p", bufs=2))
        spp = ctx.enter_context(tc.tile_pool(name="spp", bufs=2))
        sng = ctx.enter_context(tc.tile_pool(name="sng", bufs=2))
        entp = ctx.enter_context(tc.tile_pool(name="entp", bufs=2))
        catp = ctx.enter_context(tc.tile_pool(name="catp", bufs=2))
        accb = ctx.enter_context(tc.tile_pool(name="accb", bufs=2))
        prodp = ctx.enter_context(tc.tile_pool(name="prodp", bufs=4))
        smal = ctx.enter_context(tc.tile_pool(name="smal", bufs=2))
        outp = ctx.enter_context(tc.tile_pool(name="outp", bufs=1))

        constb = const.tile([128, CONSTB_N], BF16)
        nc.sync.dma_start(constb[:, 0:OFF_MASK], cb_d[:, 0:OFF_MASK])
        nc.sync.dma_start(constb[:, OFF_MASK:], cb_d[:, OFF_MASK:])
        constf = const.tile([128, 4], FP32)
        nc.sync.dma_start(constf[:], cf_d)

        wenc2 = constb[:, OFF_WENC2:OFF_WENC2 + 128]
        wdec2 = constb[:, OFF_WDEC2:OFF_WDEC2 + 128]
        wagg5 = constb[:, OFF_WAGG5:OFF_WAGG5 + 50].rearrange(
            "p (g m) -> p g m", g=5)
        convw = constb[:, OFF_CONVW:OFF_CONVW + 576].rearrange(
            "p (n c) -> p n c", n=9)
        w2v = constb[:, OFF_W2V:OFF_W2V + 128]
        w2a = constb[:, OFF_W2A:OFF_W2A + 128]
        maskpm = constb[:, OFF_MASK:OFF_MASK + H * 16].rearrange(
            "p (h n) -> p h n", h=H)
        bsum = constf[:, 0:1]          # per-partition, duplicated both halves
        bconv = constf[0:64, 1:2]
        battn2 = constf[:, 2:3]        # b_attn duplicated both halves
        baggb = constf[:, 3:4]         # b_agg replicated on all partitions

        decp = big.tile([DEC, HP, WP], BF16)
        for dli in range(4):
            dr0 = dli * (HP // 4)
            dr1 = HP if dli == 3 else (dli + 1) * (HP // 4)
            nc.sync.dma_start(decp[:, dr0:dr1, :], decp_d[:, dr0:dr1, :])

        kpad2b = big.tile([128, HP, WP], BF16, tag="kpad2b")
        # pads: half1 (k at col off 1): cols 0, 129; half2 (col off 0):
        # cols 128, 129; rows 0, 129 both halves.
        nc.gpsimd.memset(kpad2b[0:64, :, 0:1], 0.0)
        nc.gpsimd.memset(kpad2b[0:64, :, WP - 1:WP], 0.0)
        nc.gpsimd.memset(kpad2b[64:128, :, WP - 2:WP], 0.0)
        nc.gpsimd.memset(kpad2b[:, 0:1, :], 0.0)
        nc.gpsimd.memset(kpad2b[:, HP - 1:HP, :], 0.0)

        # score staging: persistent pair, rows 10:15 stay -100 forever
        sc_sbs = [big.tile([16, RC * W], BF16, tag=f"sc_sb{i}",
                           name=f"sc_sb{i}")
                  for i in range(2)]
        for t in sc_sbs:
            nc.gpsimd.memset(t[:], -100.0)

        NKG = H // KG  # 32 groups of 4 rows

        def emit_k_group(g, psk):
            esb = encp.tile([ENC, KG * W], BF16, tag="esb")
            nc.sync.dma_start(
                esb[:], encf_d[:, ENCF_PAD + g * KG * W:ENCF_PAD + (g + 1) * KG * W])
            kp = psk.tile([128, KG * W], FP32, tag="kqp")
            nc.tensor.matmul(kp[:], wenc2[0:64, :], esb[:], start=True, stop=True)
            kv1 = kp[0:64].rearrange("c (r w) -> c r w", r=KG)
            kv2 = kp[64:128].rearrange("c (r w) -> c r w", r=KG)
            r1 = 1 + g * KG
            nc.scalar.activation(
                kpad2b[0:64, r1:r1 + KG, 1:WP - 1], kv1, AF.Copy)
            nc.vector.tensor_copy(
                kpad2b[64:128, r1:r1 + KG, 0:WP - 2], kv2)

        def emit_q_group(g, psk, q2c):
            lr = (g % (RC2 // KG)) * KG  # local row inside pair-chunk tile
            qp = psk.tile([128, KG * W], FP32, tag="kqp")
            nc.tensor.matmul(
                qp[:], wdec2,
                decp[:, 1 + g * KG:1 + (g + 1) * KG, 1:WP - 1], start=True, stop=True)
            nc.scalar.activation(
                q2c[:, lr:lr + KG, :],
                qp[:].rearrange("c (r w) -> c r w", r=KG),
                AF.Identity, bias=bsum)

        psk = ctx.enter_context(
            tc.tile_pool(name="psk", bufs=2, space=bass.MemorySpace.PSUM))
        psd = ctx.enter_context(
            tc.tile_pool(name="psd", bufs=1, space=bass.MemorySpace.PSUM))
        psc = ctx.enter_context(
            tc.tile_pool(name="psc", bufs=2, space=bass.MemorySpace.PSUM))
        psf = ctx.enter_context(
            tc.tile_pool(name="psf", bufs=1, space=bass.MemorySpace.PSUM))

        k_emitted = 0
        # chunk 0 front needs k rows -1..17 -> groups 0..4 (+pair q rows 32)
        while k_emitted < 9:
            emit_k_group(k_emitted, psk)
            k_emitted += 1

        st = {}  # per-chunk front-stage tiles

        def emit_front(ch):
            nonlocal k_emitted
            r0 = ch * RC
            while k_emitted < min(NKG, 4 * ch + 13):
                emit_k_group(k_emitted, psk)
                k_emitted += 1
            if ch % 2 == 0:
                q2c = q2p.tile([128, RC2, W], BF16, tag="q2c")
                for gl in range(RC2 // KG):
                    emit_q_group(ch * (RC // KG) + gl, psk, q2c)
                st[ch] = {"q2c": q2c}
                st[ch + 1] = {"q2c": q2c}
            else:
                q2c = st[ch]["q2c"]
            lr0 = (ch % 2) * RC
            # pair adds (DVE, 16 rows) + tanh
            pair_tiles = []
            for i, d in enumerate((-1, 0, 1)):
                sp = spp.tile([128, RC, W], BF16, tag=f"sp{i}", name=f"sp{i}")
                nc.vector.tensor_tensor(
                    sp[:], q2c[:, lr0:lr0 + RC, :],
                    kpad2b[:, 1 + r0 + d:1 + r0 + RC + d, 0:W], ALU.add)
                pair_tiles.append(sp)
            spsng = sng.tile([128, RC, W], BF16, tag="spsng")
            nc.vector.tensor_tensor(
                spsng[0:64], q2c[0:64, lr0:lr0 + RC, :],
                kpad2b[0:64, 1 + r0 - 1:1 + r0 + RC - 1, 2:WP], ALU.add)
            nc.vector.tensor_tensor(
                spsng[64:128], q2c[0:64, lr0:lr0 + RC, :],
                kpad2b[0:64, 1 + r0:1 + r0 + RC, 2:WP], ALU.add)
            sp4 = sng.tile([64, RC, W], BF16, tag="sp4")
            nc.vector.tensor_tensor(
                sp4[:], q2c[0:64, lr0:lr0 + RC, :],
                kpad2b[0:64, 1 + r0 + 1:1 + r0 + RC + 1, 2:WP], ALU.add)
            entv = []
            for vi, dc in enumerate((-1, 0, 1)):
                ev = entp.tile([128, RC + 2, ENC], BF16, tag=f"ent{vi}",
                               name=f"ent{vi}")
                nc.sync.dma_start_transpose(
                    ev[:],
                    encf_d[:, ENCF_PAD + dc + (r0 - 1) * W:
                           ENCF_PAD + dc + (r0 + RC + 1) * W])
                entv.append(ev)
            st[ch].update(pair=pair_tiles, spsng=spsng, sp4=sp4, entv=entv)

        def emit_tanh(ch):
            for t in st[ch]["pair"] + [st[ch]["spsng"], st[ch]["sp4"]]:
                nc.scalar.activation(
                    t[:].rearrange("p r w -> p (r w)"),
                    t[:].rearrange("p r w -> p (r w)"), AF.Tanh)

        def emit_attn(ch):
            r0 = ch * RC
            lr0 = (ch % 2) * RC
            pair_tiles = st[ch]["pair"]
            spsng, sp4, entv = st[ch]["spsng"], st[ch]["sp4"], st[ch]["entv"]
            if ch % 2 == 0:
                st[ch]["acc"] = accb.tile([128, RC2, ENC], BF16, tag="acc_blk",
                                          name="acc_blk")
                st[ch + 1] = {**st[ch + 1], "acc": st[ch]["acc"]}
            acc_blk = st[ch]["acc"]
            # dots
            sc_sb = sc_sbs[ch % 2]
            for hc in range(4):
                dps = psd.tile([10, 4 * W], FP32, tag="dps")
                rl = hc * 4
                for g in range(3):
                    nc.tensor.matmul(
                        dps[:], wagg5[:, g, :],
                        pair_tiles[g][:, rl:rl + 4, :],
                        start=(g == 0), stop=False)
                nc.tensor.matmul(
                    dps[:], wagg5[:, 3, :], spsng[:, rl:rl + 4, :],
                    start=False, stop=False)
                nc.tensor.matmul(
                    dps[:], wagg5[0:64, 4, :], sp4[:, rl:rl + 4, :],
                    start=False, stop=True)
                nc.scalar.activation(
                    sc_sb[0:10, hc * 4 * W:(hc + 1) * 4 * W], dps[:], AF.Copy)
            # transpose + softmax
            e_pm = smal.tile([128, RC, 16], BF16, tag="e_pm")
            nc.sync.dma_start_transpose(e_pm[:], sc_sb[:])
            nc.scalar.activation(
                e_pm[:].rearrange("p r n -> p (r n)"),
                e_pm[:].rearrange("p r n -> p (r n)"), AF.Exp, bias=baggb)
            nc.vector.tensor_tensor(
                e_pm[:], e_pm[:], maskpm[:, r0:r0 + RC, :], ALU.mult)
            zs = smal.tile([128, RC], FP32, tag="zs")
            nc.vector.tensor_reduce(
                out=zs[:], in_=e_pm[:], axis=mybir.AxisListType.X, op=ALU.add)
            zr = smal.tile([128, RC], FP32, tag="zr")
            nc.vector.reciprocal(zr[:], zs[:])
            e_pm2 = smal.tile([128, RC, 16, 2], BF16, tag="e_pm2")
            nc.vector.tensor_tensor(
                e_pm2[:],
                e_pm[:].unsqueeze(3).broadcast_to([128, RC, 16, 2]),
                zr[:].unsqueeze(2).unsqueeze(3).broadcast_to([128, RC, 16, 2]),
                ALU.mult)
            # MAC
            a4 = acc_blk.rearrange("p h (a b) -> p h a b", b=2)
            macord = [3] + [j for j in range(9) if j != 3]
            for j in macord:
                dr, dc = OFFS[NMAP[j]]
                src = entv[dc + 1][:, 1 + dr:1 + dr + RC, :].rearrange(
                    "p r (a b) -> p r a b", b=2)
                wsl = e_pm2[:, :, j:j + 1, :].broadcast_to(
                    [128, RC, ENC // 2, 2])
                if j == 3:
                    nc.vector.tensor_tensor(
                        a4[:, lr0:lr0 + RC], wsl, src, ALU.mult)
                elif MAC_DMA_ACCUM[0]:
                    prod = prodp.tile([128, RC, ENC // 2, 2], BF16, tag="prod")
                    nc.vector.tensor_tensor(prod[:], wsl, src, ALU.mult)
                    nc.gpsimd.dma_start(
                        acc_blk[:, lr0:lr0 + RC, :],
                        prod[:].rearrange("p r a b -> p r (a b)"),
                        accum_op=ALU.add)
                else:
                    prod = prodp.tile([128, RC, ENC // 2, 2], BF16, tag="prod")
                    nc.vector.tensor_tensor(prod[:], wsl, src, ALU.mult)
                    nc.vector.tensor_tensor(
                        a4[:, lr0:lr0 + RC], a4[:, lr0:lr0 + RC], prod[:],
                        ALU.add)
        def emit_conv(ch):
            r0 = ch * RC
            if ch % 2 == 0:
                st[ch]["vals"] = catp.tile([128, RC, W], BF16, tag="vals_pc",
                                           name="vals_pc")
                st[ch + 1]["vals"] = st[ch]["vals"]
            vals_pc = st[ch]["vals"]
            for wpair in range(2):
                cp = psc.tile([ENC, 8, W], FP32, tag="cp")
                for half in range(2):
                    wr0 = r0 + wpair * 8 + half * 4
                    for n, (dr, dc) in enumerate(OFFS):
                        nc.tensor.matmul(
                            cp[:, half * 4:half * 4 + 4, :].rearrange(
                                "c r w -> c (r w)"),
                            convw[:, n, :],
                            decp[:, 1 + wr0 + dr:1 + wr0 + 4 + dr,
                                 1 + dc:1 + W + dc],
                            start=(n == 0), stop=(n == 8))
                lh2 = ((ch % 2) * RC + wpair * 8) // 2
                nc.scalar.activation(
                    vals_pc[0:64, lh2:lh2 + 4, :], cp[:, 0::2, :],
                    AF.Identity, bias=bconv)
                nc.scalar.activation(
                    vals_pc[64:128, lh2:lh2 + 4, :], cp[:, 1::2, :],
                    AF.Identity, bias=bconv)

        def emit_final(b):
            ch = 2 * b + 1
            acc_blk = st[ch]["acc"]
            vals_pc = st[ch]["vals"]
            b0 = b * RC2
            attn_pc = catp.tile([128, RC, W], BF16, tag="attn_pc")
            nc.sync.dma_start_transpose(
                attn_pc[:], acc_blk[:].rearrange("p h c -> p (h c)"))
            outsb = outp.tile([ENC, RC2, W], BF16, tag="outsb")
            for wi in range(RC // 4):
                fp = psf.tile([128, 4, W], FP32, tag="fp")
                fpf = fp[:].rearrange("c r w -> c (r w)")
                nc.tensor.matmul(
                    fpf, w2v, vals_pc[:, wi * 4:(wi + 1) * 4, :],
                    start=True, stop=False)
                nc.tensor.matmul(
                    fpf, w2a, attn_pc[:, wi * 4:(wi + 1) * 4, :],
                    start=False, stop=True)
                ob0 = wi * 8
                tll = smal.tile([128, 4, W], BF16, tag="tll")
                nc.scalar.activation(tll[:], fp[:], AF.Identity, bias=battn2)
                nc.vector.scalar_tensor_tensor(
                    outsb[:, ob0:ob0 + 8:2, :], tll[0:64], 0.2,
                    tll[0:64], ALU.mult, ALU.max)
                nc.vector.scalar_tensor_tensor(
                    outsb[:, ob0 + 1:ob0 + 8:2, :], tll[64:128], 0.2,
                    tll[64:128], ALU.mult, ALU.max)
            nc.sync.dma_start(out_d[:, b0:b0 + RC2, :], outsb[:])
            del st[ch - 1], st[ch]

        CLAG = 2
        emit_front(0)
        emit_tanh(0)
        for ch in range(NCH):
            if ch + 1 < NCH:
                emit_front(ch + 1)
            emit_attn(ch)
            if ch - CLAG >= 0:
                emit_conv(ch - CLAG)
                if (ch - CLAG) % 2 == 1:
                    emit_final((ch - CLAG) // 2)
            if ch + 1 < NCH:
                emit_tanh(ch + 1)
        for ch in range(NCH - CLAG, NCH):
            emit_conv(ch)
            if ch % 2 == 1:
                emit_final(ch // 2)

    nc.compile()
    return nc


_PROG = None
_RUN_KWARGS = {}
_LAST_RESULT = None


def _get_prog():
    global _PROG
    if _PROG is None:
        _PROG = build_program()
    return _PROG


def _make_mask_pm():
    """[W(part), H, 16] validity mask in NMAP column order."""
    m = np.zeros((W, H, 16), dtype=np.float32)
    for j, n in enumerate(NMAP):
        dr, dc = OFFS[n]
        rv = np.arange(H) + dr
        cv = np.arange(W) + dc
        m[:, :, j] = (((cv >= 0) & (cv < W))[:, None]
                      & ((rv >= 0) & (rv < H))[None, :]).astype(np.float32)
    return m


def _pack_constb(W_dec, W_enc, W_agg, conv_w, W_attn):
    bf = ml_dtypes.bfloat16
    cb = np.zeros((128, CONSTB_N), dtype=np.float32)
    we = np.asarray(W_enc, np.float32)
    cb[0:64, OFF_WENC2:OFF_WENC2 + 64] = we
    cb[0:64, OFF_WENC2 + 64:OFF_WENC2 + 128] = we
    wd = np.asarray(W_dec, np.float32)
    cb[:, OFF_WDEC2:OFF_WDEC2 + 64] = wd
    cb[:, OFF_WDEC2 + 64:OFF_WDEC2 + 128] = wd
    wa = np.asarray(W_agg, np.float32)[:, 0]
    w5 = np.zeros((128, 5, 10), dtype=np.float32)
    for g in range(4):  # groups 0-2 pairs, group 3 stacked singles
        w5[0:64, g, 2 * g] = wa
        w5[64:128, g, 2 * g + 1] = wa
    w5[0:64, 4, 8] = wa
    cb[:, OFF_WAGG5:OFF_WAGG5 + 50] = w5.reshape(128, 50)
    cw = np.asarray(conv_w, np.float32).reshape(9, DEC, ENC).transpose(1, 0, 2)
    cb[:, OFF_CONVW:OFF_CONVW + 576] = cw.reshape(128, 576)
    wat = np.asarray(W_attn, np.float32)
    for par in range(2):  # block-diag: row parity stays separated
        sl = slice(par * 64, par * 64 + 64)
        cb[sl, OFF_W2V + par * 64:OFF_W2V + par * 64 + 64] = wat[0:64]
        cb[sl, OFF_W2A + par * 64:OFF_W2A + par * 64 + 64] = wat[64:128]
    cb[:, OFF_MASK:OFF_MASK + H * 16] = _make_mask_pm().reshape(128, H * 16)
    return cb.astype(bf)


def kernel(encoder_features, decoder_features, W_enc, b_enc, W_dec, b_dec,
           W_agg, b_agg, W_attn, b_attn, conv_w, conv_b):
    bf = ml_dtypes.bfloat16
    nc = _get_prog()

    cf = np.zeros((128, 4), dtype=np.float32)
    bs = np.asarray(b_dec, np.float32) + np.asarray(b_enc, np.float32)
    cf[0:64, 0] = bs
    cf[64:128, 0] = bs
    cf[0:64, 1] = np.asarray(conv_b, np.float32)
    cf[0:64, 2] = np.asarray(b_attn, np.float32)
    cf[64:128, 2] = np.asarray(b_attn, np.float32)
    cf[:, 3] = float(np.asarray(b_agg).reshape(-1)[0])

    shared = {
        "constb": _pack_constb(W_dec, W_enc, W_agg, conv_w, W_attn),
        "constf": cf,
    }
    enc_all = np.asarray(encoder_features, np.float32).astype(bf)
    dec_all = np.asarray(decoder_features, np.float32).astype(bf)
    in_maps = []
    for c in range(N_CORES):
        encf = np.zeros((ENC, ENCF_N), dtype=bf)
        encf[:, ENCF_PAD:ENCF_PAD + H * W] = enc_all[c].reshape(ENC, H * W)
        decp = np.zeros((DEC, HP, WP), dtype=bf)
        decp[:, 1:HP - 1, 1:WP - 1] = dec_all[c]
        m = dict(shared)
        m["encf"] = encf
        m["decp"] = decp
        in_maps.append(m)

    res = run_bass_kernel_spmd(nc, in_maps, list(range(N_CORES)),
                               **_RUN_KWARGS)
    global _LAST_RESULT
    _LAST_RESULT = res
    out = np.stack(
        [np.asarray(res.results[c]["out"], np.float32) for c in range(N_CORES)])
    return out
